# revision 8
# baseline (speedup 1.0000x reference)
"""DicePolyTopk loss kernel for trn2 (8 NeuronCores, SPMD data-parallel).

Math: out = dice_loss + mean(top_k(poly1, k)) with
  bce   = -(t*log(i) + (1-t)*log1p(-i))
  poly1 = bce + eps*(1 - exp(-bce))          (monotone increasing in bce)
  k     = 10% of N,  N = 64*512*512 = 16,777,216

Because poly1 is monotone in bce, the top-k of poly1 is the top-k of bce.
Host picks a threshold beta ~= k-th largest bce from a strided sample; each
core then computes exact masked sums via clamped reductions:
  T1 = sum(min(-bce, -beta))        -> sum of bce over selected + count terms
  T2 = sum(exp(min(-bce, -beta)))   -> sum of pt=exp(-bce) over selected
  C  = #{bce > beta}
  SI = sum(i), ST = sum(t), SIT = sum(i*t)   (dice terms, via TensorE)
and the host combines with the variational correction
  topk_sum = sum_{bce>beta} poly1 + (k - C) * poly1(beta)
which is exact when beta equals the true k-th value and second-order
insensitive (O(rho * (beta_err)^2)) otherwise.

Per-core engine split (2,097,152 elems as [128, 16384], 8 chunks of 2048):
  ScalarE: L1=ln(i), L2=ln(1-i), E=exp(cl) (+fused accum for T2)
  VectorE: P=t*D, bq=L2+P, cl=min(bq,-beta) (+accum T1), cnt=is_lt (+accum C)
  GpSimd : D = L1 - L2
  TensorE: SI,ST via ones-matmul; SIT via diagonal of i_chunk.T @ t_chunk
"""

import numpy as np
from contextlib import ExitStack

from concourse import bass, bacc, mybir
from concourse import tile
from concourse.bass_utils import run_bass_kernel_spmd

P = 128
FREE = 16384            # per-core free dim -> 2,097,152 elems/core
CHUNK = 2048
NCHUNK = FREE // CHUNK  # 8
NCORES = 8
N_TOTAL = 64 * 512 * 512
K_TOP = int(N_TOTAL * 10 / 100)
EPS_POLY = 3.1
SMOOTH = 1.0

F32 = mybir.dt.float32
AF = mybir.ActivationFunctionType
OP = mybir.AluOpType


def build_program():
    nc = bacc.Bacc("TRN2", target_bir_lowering=False, debug=False,
                   num_devices=NCORES)

    preds = nc.dram_tensor("preds", [P, FREE], F32, kind="ExternalInput").ap()
    gts = nc.dram_tensor("gt", [P, FREE], F32, kind="ExternalInput").ap()
    thr = nc.dram_tensor("thr", [P, 1], F32, kind="ExternalInput").ap()

    o_t1 = nc.dram_tensor("accT1", [P, NCHUNK], F32, kind="ExternalOutput").ap()
    o_t2 = nc.dram_tensor("accT2", [P, NCHUNK], F32, kind="ExternalOutput").ap()
    o_c = nc.dram_tensor("accC", [P, NCHUNK], F32, kind="ExternalOutput").ap()
    o_s = nc.dram_tensor("sums2", [2, 512], F32, kind="ExternalOutput").ap()
    o_it = nc.dram_tensor("psit", [P, 128], F32, kind="ExternalOutput").ap()

    with tile.TileContext(nc) as tc, ExitStack() as ctx:
        pool = ctx.enter_context(tc.tile_pool(name="work", bufs=2))
        cpool = ctx.enter_context(tc.tile_pool(name="consts", bufs=1))
        apool = ctx.enter_context(tc.tile_pool(name="accs", bufs=1))
        pp = ctx.enter_context(tc.tile_pool(name="ps", bufs=1, space="PSUM"))

        ones = cpool.tile([P, 1], F32, tag="ones")
        nc.vector.memset(ones[:], 1.0)
        thr_sb = cpool.tile([P, 1], F32, tag="thr")
        nc.sync.dma_start(thr_sb[:], thr)

        accT1 = apool.tile([P, NCHUNK], F32, tag="aT1")
        accT2 = apool.tile([P, NCHUNK], F32, tag="aT2")
        accC = apool.tile([P, NCHUNK], F32, tag="aC")

        ps_i = pp.tile([1, 512], F32, tag="psi")
        ps_t = pp.tile([1, 512], F32, tag="pst")
        ps_it = pp.tile([P, 128], F32, tag="psit")
        ps_dummy = pp.tile([1, 1], F32, tag="psd")

        # Priming matmul: absorbs the cross-engine wait on the ones-memset so
        # the per-chunk matmuls carry a single DMA wait (the LDWEIGHTS slot
        # only fits one sync-wait command).
        nc.tensor.matmul(ps_dummy[:], ones[:], ones[:], start=True, stop=True,
                         skip_group_check=True)

        n512 = CHUNK // 512
        n128 = CHUNK // 128

        for c in range(NCHUNK):
            sl = bass.ts(c, CHUNK)
            ti = pool.tile([P, CHUNK], F32, tag="i")
            tt = pool.tile([P, CHUNK], F32, tag="t")
            nc.sync.dma_start(ti[:], preds[:, sl])
            nc.sync.dma_start(tt[:], gts[:, sl])

            l1 = pool.tile([P, CHUNK], F32, tag="l1")
            nc.scalar.activation(l1[:], ti[:], AF.Ln)
            l2 = pool.tile([P, CHUNK], F32, tag="l2")
            nc.scalar.activation(l2[:], ti[:], AF.Ln, bias=1.0, scale=-1.0)

            d = pool.tile([P, CHUNK], F32, tag="d")
            nc.gpsimd.tensor_tensor(d[:], l1[:], l2[:], OP.subtract)

            pmul = pool.tile([P, CHUNK], F32, tag="p")
            nc.vector.tensor_tensor(pmul[:], tt[:], d[:], OP.mult)
            bq = pool.tile([P, CHUNK], F32, tag="bq")
            nc.vector.tensor_tensor(bq[:], l2[:], pmul[:], OP.add)

            cl = pool.tile([P, CHUNK], F32, tag="cl")
            nc.vector.tensor_scalar(cl[:], bq[:], thr_sb[:], None, OP.min,
                                    OP.add, accum_out=accT1[:, c:c + 1])
            ex = pool.tile([P, CHUNK], F32, tag="ex")
            nc.scalar.activation(ex[:], cl[:], AF.Exp,
                                 accum_out=accT2[:, c:c + 1])
            cnt = pool.tile([P, CHUNK], F32, tag="cnt")
            nc.vector.tensor_scalar(cnt[:], bq[:], thr_sb[:], None, OP.is_lt,
                                    OP.add, accum_out=accC[:, c:c + 1])

            for s in range(n512):
                first = (c == 0 and s == 0)
                last = (c == NCHUNK - 1 and s == n512 - 1)
                nc.tensor.matmul(ps_i[:], ones[:], ti[:, bass.ts(s, 512)],
                                 start=first, stop=last, skip_group_check=True)
                nc.tensor.matmul(ps_t[:], ones[:], tt[:, bass.ts(s, 512)],
                                 start=first, stop=last, skip_group_check=True)
            for j in range(n128):
                first = (c == 0 and j == 0)
                last = (c == NCHUNK - 1 and j == n128 - 1)
                nc.tensor.matmul(ps_it[:], ti[:, bass.ts(j, 128)],
                                 tt[:, bass.ts(j, 128)],
                                 start=first, stop=last, skip_group_check=True)

        si = cpool.tile([1, 512], F32, tag="si")
        nc.vector.tensor_copy(si[:], ps_i[:])
        st = cpool.tile([1, 512], F32, tag="st")
        nc.vector.tensor_copy(st[:], ps_t[:])
        cit = cpool.tile([P, 128], F32, tag="cit")
        nc.vector.tensor_copy(cit[:], ps_it[:])

        nc.sync.dma_start(o_s[0:1, :], si[:])
        nc.sync.dma_start(o_s[1:2, :], st[:])
        nc.sync.dma_start(o_it, cit[:])
        nc.sync.dma_start(o_t1, accT1[:])
        nc.sync.dma_start(o_t2, accT2[:])
        nc.sync.dma_start(o_c, accC[:])

    nc.compile()
    return nc


_NC = None


def _get_nc():
    global _NC
    if _NC is None:
        _NC = build_program()
    return _NC


def _pick_beta(p_flat, t_flat):
    """Sample quantile estimate of the k-th largest bce value."""
    ps = p_flat[::16].astype(np.float64)
    ts = t_flat[::16].astype(np.float64)
    bce = -(ts * np.log(ps) + (1.0 - ts) * np.log1p(-ps))
    m = bce.size
    ks = max(1, int(round(K_TOP / N_TOTAL * m)))
    return float(np.partition(bce, m - ks)[m - ks])


def _prepare(preds, gt_masks):
    p_flat = np.ascontiguousarray(np.asarray(preds, dtype=np.float32).reshape(-1))
    t_flat = np.ascontiguousarray(np.asarray(gt_masks, dtype=np.float32).reshape(-1))
    assert p_flat.size == N_TOTAL

    beta = _pick_beta(p_flat, t_flat)
    thr_np = np.full((P, 1), np.float32(-beta), dtype=np.float32)

    per_core = N_TOTAL // NCORES
    in_maps = []
    for c in range(NCORES):
        pc = p_flat[c * per_core:(c + 1) * per_core].reshape(P, FREE)
        tc_ = t_flat[c * per_core:(c + 1) * per_core].reshape(P, FREE)
        in_maps.append({"preds": pc, "gt": tc_, "thr": thr_np})
    return in_maps, beta


def _combine(results, beta):
    T1 = 0.0
    T2 = 0.0
    C = 0.0
    SI = 0.0
    ST = 0.0
    SIT = 0.0
    for r in results:
        T1 += float(r["accT1"].astype(np.float64).sum())
        T2 += float(r["accT2"].astype(np.float64).sum())
        C += float(r["accC"].astype(np.float64).sum())
        SI += float(r["sums2"][0].astype(np.float64).sum())
        ST += float(r["sums2"][1].astype(np.float64).sum())
        SIT += float(np.trace(r["psit"].astype(np.float64)))

    C = round(C)
    eb = np.exp(-beta)
    # T1 = sum(min(-x, -beta)) = -(sum_{x>beta} x + (N - C)*beta)
    sum_x_sel = -T1 - (N_TOTAL - C) * beta
    # T2 = sum(exp(min(-x,-beta))) = sum_{x>beta} e^-x + (N - C)*e^-beta
    sum_pt_sel = T2 - (N_TOTAL - C) * eb
    a_sel = sum_x_sel + EPS_POLY * C - EPS_POLY * sum_pt_sel
    poly_beta = beta + EPS_POLY * (1.0 - eb)
    topk_sum = a_sel + (K_TOP - C) * poly_beta
    topk_mean = topk_sum / K_TOP

    dice = 1.0 - (2.0 * SIT + SMOOTH) / (SI + ST + SMOOTH)
    return np.float32(dice + topk_mean)


def run(preds, gt_masks, trace=False):
    """Returns (scalar_result, BassKernelResults)."""
    nc = _get_nc()
    in_maps, beta = _prepare(preds, gt_masks)
    res = run_bass_kernel_spmd(nc, in_maps, core_ids=list(range(NCORES)),
                               trace=trace)
    out = _combine(res.results, beta)
    return out, res


def kernel(preds, gt_masks):
    out, _ = run(preds, gt_masks, trace=False)
    return np.array(out, dtype=np.float32)
